# revision 11
# baseline (speedup 1.0000x reference)
"""Block-sparse matmul kernel for Trainium2 (8 NeuronCores, SPMD v2).

out = relu(x @ W_sparse + bias);  x [1024,4096], 4096 active 32x32
blocks at (ci, co) in a 128x128 block grid, bias [4096].

Strategy: exploit block sparsity on the PE via 64x32 array tiling.
Per-core share: co-quarter (32 block-cols) x batch-half (512 rows).
Each weight block runs as part of a [K<=64, M=32, N=512] matmul on a
64x32 sub-tile of the PE at tile_position (64H, 32C); blocks of the
same co that land in the same (ktile, row-half) cell are PAIRED into
one instruction (host-side max-weight matching chooses the ci ->
(ktile, band) layout to maximize pairs).  Sustained PE rate is set by
the LDWEIGHTS stream (~35ns per 32-col load), so cells (not blocks)
are the unit of cost: ~1024 blocks collapse to ~750 cells/core.

The 4 co-quarters have different sparsity structures, so one SPMD
program branches with tc.Switch(partition_id % 4, 4) into 4 exact
per-quarter schedules (no padding).  Tile allocations are shared
outside the Switch; PSUM is a single [128, 4096] f32 tile (all 8
banks) so arms may touch it in any order (the control-flow merge
checker requires identical TILE-touch order across arms, not
identical instructions).  Accumulation: psum slot per (co, H) =
[32, 512] at bank 2*(g%4)+H, partitions 32C; 2 phases of 4
co-groups; cells round-robin the 8 tile positions (stream ~213ns vs
~30ns issue) with kt-ascending order in phase 0 to chase the x DMA
stream.  Evict: ACT copy -> DVE add -> ACT relu+bias -> DMA, few
full-width ops (each engine PSUM access disturbs MM streams).
DMA descriptor issue costs ~3.6us/MB of sequencer time, so the
stream is split: head chunks pre-Switch on the sync (x) and scalar
(W+bias) HWDGE rings sized to the arm-entry rendezvous, the rest
inside arms; switch_hint() starts arm-code prefetch early.
"""

import numpy as np
import ml_dtypes

import concourse.bacc as bacc
import concourse.mybir as mybir
import concourse.tile as tile
from concourse.bass_utils import run_bass_kernel_spmd

BS = 32
N_IN = 4096
N_OUT = 4096
BATCH = 1024
N_CORES = 8
NKT = 32                 # ktiles (x feature column tiles of 128)
B_PER_CORE = 512
BF16 = mybir.dt.bfloat16
F32 = mybir.dt.float32

_CACHE = {}


# ----------------------------------------------------------------- host prep

def _snake(items, n):
    """Distribute items (ordered) into n bins, snake order."""
    bins = [[] for _ in range(n)]
    for rank, it in enumerate(items):
        r = rank % (2 * n)
        b = r if r < n else 2 * n - 1 - r
        bins[b].append(it)
    return bins


def _structure(ci, co):
    """Build per-quarter schedules from the sparsity pattern."""
    ci = np.asarray(ci, np.int64)
    co = np.asarray(co, np.int64)
    n_co = np.bincount(co, minlength=128)
    order = list(np.argsort(-n_co, kind="stable"))
    quarters = _snake(order, 4)

    structs = []
    for q in range(4):
        cos = quarters[q]
        co_set = set(int(c) for c in cos)
        bidx = np.flatnonzero(np.isin(co, cos))

        # co -> (group g, band C)
        cs = sorted(cos, key=lambda c: -n_co[c])
        groups = _snake(cs, 8)
        slot = {}
        for g in range(8):
            assert len(groups[g]) == 4
            for C, c in enumerate(groups[g]):
                slot[int(c)] = (g, C)

        # ci-pair matching to maximize 2-block cells
        co_idx = {int(c): j for j, c in enumerate(cos)}
        A = np.zeros((128, 32), np.int32)
        for b in bidx:
            A[ci[b], co_idx[int(co[b])]] = 1
        W = A @ A.T
        np.fill_diagonal(W, -1)
        flat = np.argsort(-W, axis=None, kind="stable")
        used = np.zeros(128, bool)
        pairs = []
        for f in flat:
            a, b2 = divmod(int(f), 128)
            if a < b2 and not used[a] and not used[b2]:
                used[a] = used[b2] = True
                pairs.append((a, b2))
                if len(pairs) == 64:
                    break
        pos = {}
        for j, (a, b2) in enumerate(pairs):
            kt, H = j % 32, j // 32
            pos[a] = (kt, 2 * H)
            pos[b2] = (kt, 2 * H + 1)
        assert len(pos) == 128

        # cells
        cells = {}
        for b in bidx:
            a, c = int(ci[b]), int(co[b])
            kt, band = pos[a]
            H = band // 2
            cells.setdefault((c, kt, H), []).append((int(b), band))

        recs = []
        have = set()
        for (c, kt, H), bl in cells.items():
            g, C = slot[c]
            recs.append(dict(g=g, C=C, H=H, kt=kt, blocks=bl, co=c))
            have.add((g, C, H))
        for g in range(8):
            for C in range(4):
                for H in range(2):
                    if (g, C, H) not in have:
                        recs.append(dict(g=g, C=C, H=H, kt=0, blocks=[],
                                         co=groups[g][C]))
        # phase (groups 0-3 then 4-7).  Phase 0: kt-ascending (x-stream
        # gating) with round-robin over the 8 (H,C) tile positions so
        # consecutive instructions never hit the same sub-tile (stream
        # is ~213ns; issue ~30ns; 8-way spacing avoids head-of-line
        # stalls).  Phase 1: group-major so evictions stagger, with the
        # same position round-robin inside each group.
        def rr_interleave(cells_list):
            buckets = {}
            for r in sorted(cells_list, key=lambda r: r["kt"]):
                buckets.setdefault((r["H"], r["C"]), []).append(r)
            seqs = [buckets[k]
                    for k in sorted(buckets, key=lambda k: (k[1], k[0]))]
            out = []
            while seqs:
                for s in seqs:
                    out.append(s.pop(0))
                seqs = [s for s in seqs if s]
            return out

        p0 = [r for r in recs if r["g"] < 4]
        p1 = [r for r in recs if r["g"] >= 4]
        events = [("mm", r) for r in rr_interleave(p0)]
        events += [("evict", g) for g in range(4)]
        for g in range(4, 8):
            events += [("mm", r) for r in
                       rr_interleave([r for r in p1 if r["g"] == g])]
            events.append(("evict", g))
        recs = [r for kind, r in events if kind == "mm"]

        # start/stop per (g,C,H) slot; W col per H-sequence
        first, last = {}, {}
        for i, r in enumerate(recs):
            key = (r["g"], r["C"], r["H"])
            first.setdefault(key, i)
            last[key] = i
        nh = [0, 0]
        for i, r in enumerate(recs):
            key = (r["g"], r["C"], r["H"])
            r["start"] = first[key] == i
            r["stop"] = last[key] == i
            r["wcol"] = nh[r["H"]]
            nh[r["H"]] += 1
        structs.append(dict(recs=recs, events=events, slot=slot,
                            groups=groups, pos=pos, nh=nh))
    nw = max(max(s["nh"]) for s in structs)
    return structs, nw


# ------------------------------------------------------------- bass program

def _build_program(structs, nw):
    nc = bacc.Bacc(trn_type="TRN2")
    xT_d = nc.dram_tensor("xT", [128, NKT * 512], BF16, kind="ExternalInput")
    wK_d = nc.dram_tensor("wK", [128, nw * 32], BF16, kind="ExternalInput")
    bias_d = nc.dram_tensor("biasv", [128, 8], F32, kind="ExternalInput")
    out_d = nc.dram_tensor("outT", [8, 128, 512], BF16, kind="ExternalOutput")

    WC = nw * 32

    with tile.TileContext(nc) as tc:
        with (
            tc.tile_pool(name="xp", bufs=1) as xp,
            tc.tile_pool(name="wp", bufs=1) as wp,
            tc.tile_pool(name="bp", bufs=1) as bp,
            tc.tile_pool(name="op", bufs=8) as op,
            tc.tile_pool(name="tp", bufs=4) as tp,
            tc.tile_pool(name="ps", bufs=1, space="PSUM") as ps,
        ):
            xt = xp.tile([128, NKT * 512], BF16)
            wt = wp.tile([128, WC], BF16)
            bv = bp.tile([128, 8], F32)
            pst = ps.tile([128, 4096], F32)
            obf = [op.tile([128, 512], BF16, tag="o", name=f"o{g}")
                   for g in range(8)]
            tts = [tp.tile([128, 512], F32, tag="t", name=f"t{i}")
                   for i in range(4)]

            # warmup: keep PE busy through the HAM window + DMA ramp,
            # in 64x32 mode so no mode switch before the real stream.
            # 64 zero warmup MMs (~2.6us) fit under the sync ring's
            # longer pre-switch DMA window, so they don't delay the
            # arm-entry rendezvous but carry the PE through the HAM
            # cold-clock window before the first cells stream.
            wut = obf[0]
            nc.vector.memset(wut[:], 0.0)
            for i in range(64):
                H, C = i % 2, (i // 2) % 4
                nc.tensor.matmul(
                    pst[32 * C:32 * C + 32, 512 * H:512 * H + 512],
                    wut[64 * H:64 * H + 64, 0:32],
                    wut[64 * H:64 * H + 64, 0:512],
                    start=True, stop=True,
                    tile_position=(64 * H, 32 * C),
                    skip_group_check=True,
                )

            qv = {
                mybir.EngineType.PE: nc.tensor.partition_id() % 4,
                mybir.EngineType.DVE: nc.vector.partition_id() % 4,
                mybir.EngineType.Activation: nc.scalar.partition_id() % 4,
                mybir.EngineType.SP: nc.sync.partition_id() % 4,
            }
            # Arm-code prefetch (~5us for ~740-instruction arms) starts
            # at the hint site, overlapping the pre-switch DMA issue.
            hint = nc.switch_hint(
                engines=[nc.tensor, nc.vector, nc.scalar, nc.sync],
                index=[qv[mybir.EngineType.PE],
                       qv[mybir.EngineType.DVE],
                       qv[mybir.EngineType.Activation],
                       qv[mybir.EngineType.SP]],
                n=4)

            # Pre-switch: just enough data to cover the first cells
            # (the arm-entry rendezvous costs max(own pre-switch) +
            # prefetch, so keep every engine's pre-switch short).
            nc.scalar.dma_start(bv[:], bias_d[:])
            h0 = (5 * WC // 16) & ~31
            nc.scalar.dma_start(wt[:, 0:h0 // 2], wK_d[:, 0:h0 // 2])
            nc.scalar.dma_start(wt[:, h0 // 2:h0], wK_d[:, h0 // 2:h0])
            for a, b in [(0, 1), (1, 2), (2, 4), (4, 7), (7, 10)]:
                nc.sync.dma_start(xt[:, a * 512:b * 512],
                                  xT_d[:, a * 512:b * 512])

            def evict(g):
                bank0 = 512 * (2 * (g % 4))
                bank1 = bank0 + 512
                ta = tts[2 * (g % 2)]
                tb = tts[2 * (g % 2) + 1]
                # full-width ops: every engine-side PSUM access disrupts
                # the concurrent MM streams by ~500ns, so use as few
                # PSUM-touching ops as possible (1 ACT copy + 1 DVE add).
                nc.scalar.activation(
                    ta[:], pst[:, bank0:bank0 + 512],
                    mybir.ActivationFunctionType.Copy)
                nc.vector.tensor_tensor(
                    tb[:], ta[:], pst[:, bank1:bank1 + 512],
                    mybir.AluOpType.add)
                nc.scalar.activation(
                    obf[g][:], tb[:], mybir.ActivationFunctionType.Relu,
                    bias=bv[:, g:g + 1])
                nc.sync.dma_start(out_d[g], obf[g][:])

            for q in tc.Switch(qv, 4, hint=hint):
                s = structs[q]
                # rest of the data, issued from inside the arm (identical
                # across arms); x stays on the sync ring only so each
                # cell's wait tracks exactly its own chunk.
                for a, b in [(10, 13), (13, 16), (16, 20), (20, 24),
                             (24, 28), (28, 32)]:
                    nc.sync.dma_start(xt[:, a * 512:b * 512],
                                      xT_d[:, a * 512:b * 512])
                h1, h2 = (9 * WC) // 16, (13 * WC) // 16
                nc.scalar.dma_start(wt[:, h0:h1], wK_d[:, h0:h1])
                nc.scalar.dma_start(wt[:, h1:h2], wK_d[:, h1:h2])
                nc.scalar.dma_start(wt[:, h2:WC], wK_d[:, h2:WC])
                for kind, r in s["events"]:
                    if kind == "evict":
                        evict(r)
                        continue
                    g, C, H, kt = r["g"], r["C"], r["H"], r["kt"]
                    bank = 512 * (2 * (g % 4) + H)
                    nc.tensor.matmul(
                        pst[32 * C:32 * C + 32, bank:bank + 512],
                        wt[64 * H:64 * H + 64,
                           32 * r["wcol"]:32 * r["wcol"] + 32],
                        xt[64 * H:64 * H + 64, 512 * kt:512 * kt + 512],
                        start=r["start"], stop=r["stop"],
                        tile_position=(64 * H, 32 * C),
                        skip_group_check=True,
                    )

    nc.compile()
    return nc


# ------------------------------------------------------------------ per-core

def _core_inputs(structs, nw, x, kernel, bias):
    x = np.asarray(x, np.float32)
    kernel = np.asarray(kernel, np.float32)
    bias = np.asarray(bias, np.float32)
    in_maps = []
    for cid in range(N_CORES):
        q, h = cid % 4, cid // 4
        s = structs[q]
        xh = x[512 * h:512 * h + 512]          # [512, 4096]
        # xT[32*band+p, kt*512+n] = xh[n, 32*a+p] for a at (kt, band)
        xT = np.empty((128, NKT * 512), np.float32)
        for a, (kt, band) in s["pos"].items():
            xT[32 * band:32 * band + 32, 512 * kt:512 * (kt + 1)] = \
                xh[:, 32 * a:32 * a + 32].T
        wK = np.zeros((128, nw * 32), np.float32)
        for r in s["recs"]:
            H = r["H"]
            for b, band in r["blocks"]:
                ro = 64 * H + 32 * (band - 2 * H)
                wK[ro:ro + 32, 32 * r["wcol"]:32 * (r["wcol"] + 1)] = \
                    kernel[b]
        bvv = np.zeros((128, 8), np.float32)
        for g in range(8):
            for C, c in enumerate(s["groups"][g]):
                bvv[32 * C:32 * C + 32, g] = bias[32 * c:32 * c + 32]
        in_maps.append({
            "xT": xT.astype(ml_dtypes.bfloat16),
            "wK": wK.astype(ml_dtypes.bfloat16),
            "biasv": bvv,
        })
    return in_maps


def _assemble(structs, results):
    out = np.empty((BATCH, N_OUT), np.float32)
    for cid in range(N_CORES):
        q, h = cid % 4, cid // 4
        s = structs[q]
        o = np.asarray(results[cid]["outT"], np.float32)  # [8,128,512]
        for g in range(8):
            for C, c in enumerate(s["groups"][g]):
                out[512 * h:512 * h + 512, 32 * c:32 * c + 32] = \
                    o[g, 32 * C:32 * C + 32, :].T
    return out


def _coalesce(kernel, ci, co):
    """Sum duplicate (ci,co) blocks so each grid position is unique."""
    flat = ci.astype(np.int64) * 128 + co.astype(np.int64)
    uniq, inv = np.unique(flat, return_inverse=True)
    if len(uniq) == len(flat):
        return kernel, ci, co
    k2 = np.zeros((len(uniq), BS, BS), np.float32)
    np.add.at(k2, inv, np.asarray(kernel, np.float32))
    return k2, (uniq // 128).astype(ci.dtype), (uniq % 128).astype(co.dtype)


def run(x, kernel, bias, ci, co, trace=False):
    ci = np.asarray(ci)
    co = np.asarray(co)
    kernel, ci, co = _coalesce(np.asarray(kernel), ci, co)
    key = (ci.tobytes(), co.tobytes())
    if _CACHE.get("key") != key:
        structs, nw = _structure(ci, co)
        nc = _build_program(structs, nw)
        _CACHE.update(key=key, structs=structs, nw=nw, nc=nc)
    structs, nw, nc = _CACHE["structs"], _CACHE["nw"], _CACHE["nc"]
    in_maps = _core_inputs(structs, nw, x, kernel, bias)
    last_err = None
    for attempt in range(3):
        try:
            res = run_bass_kernel_spmd(nc, in_maps,
                                       core_ids=list(range(N_CORES)),
                                       trace=trace)
            return _assemble(structs, res.results), res
        except Exception as e:
            last_err = e
            import time
            time.sleep(2.0)
    raise last_err


def kernel(x, kernel, bias, ci, co):
    out, _ = run(x, kernel, bias, ci, co, trace=False)
    return out
